# revision 1
# baseline (speedup 1.0000x reference)
"""Fastmax (p=1 causal linear attention) Trainium2 kernel, 8-core SPMD.

Sharding: data-parallel over heads (16 heads -> 2 per core). Each core
computes q/k/v projections for its 2 heads, the chunked causal linear
attention (augmented [65,65] prefix state carries S, ksum, vsum, count),
and a partial output projection. Host sums the 8 partial outputs + bias.

Layouts (per core):
  qh/kh        [65, 2048]   per-head transposed aug (row 64 = ones)
  krows/vrows  [128, 16*80] token-major aug rows (col c*80+64 = ones)
  vht          [128, 2048]  attention output, transposed, heads stacked
Scores matmul of aug q x aug k gives 1 + s*<q,k> directly; one [65,65]
PSUM state accumulates K'^T V~ across chunks; row-layout O puts the
denominator in a column so reciprocal+divide are per-partition ops.
"""

import sys

sys.path.insert(0, "/opt/trn_rl_repo")

import numpy as np

B, N, D_MODEL, H, D_HEAD = 1, 2048, 1024, 16, 64
NCORES = 8
HPC = H // NCORES  # heads per core
DPC = HPC * D_HEAD  # out dims per core (128)
CH = 128  # chunk (tokens)
SPAN = 256  # query span (2 chunks)
NSPAN = N // SPAN
KT = D_MODEL // 128  # contraction tiles for projections
NT = N // 512  # 512-wide column chunks of the sequence
RST = 80  # row-buffer stride per chunk (64 data + ones col + pad)

_CACHE = {}


def _build():
    import concourse.bass as bass
    import concourse.tile as tile
    import concourse.mybir as mybir
    from concourse import bacc
    from concourse.alu_op_type import AluOpType

    BF = mybir.dt.bfloat16
    F32 = mybir.dt.float32
    AF = mybir.ActivationFunctionType
    AX = mybir.AxisListType

    nc = bacc.Bacc("TRN2", target_bir_lowering=False, debug=False, num_devices=NCORES)

    xt_d = nc.declare_dram_parameter("xt", [D_MODEL, N], BF, isOutput=False)
    wq_d = nc.declare_dram_parameter("wq", [128, D_MODEL], BF, isOutput=False)
    wk_d = nc.declare_dram_parameter("wk", [128, D_MODEL], BF, isOutput=False)
    wvk_d = nc.declare_dram_parameter("wvk", [128, 2 * D_MODEL], BF, isOutput=False)
    wo_d = nc.declare_dram_parameter("wo", [DPC, D_MODEL], BF, isOutput=False)
    consts_d = nc.declare_dram_parameter("consts", [128, 514], BF, isOutput=False)
    onesrow_d = nc.declare_dram_parameter("onesrow", [1, N], BF, isOutput=False)
    out_d = nc.declare_dram_parameter("out", [N, D_MODEL], BF, isOutput=True)

    with tile.TileContext(nc) as tc:
        with (
            tc.tile_pool(name="const", bufs=1) as constp,
            tc.tile_pool(name="wqkv", bufs=1) as wp,
            tc.tile_pool(name="acts", bufs=1) as actp,
        ):
            consts = constp.tile([128, 514], BF)
            nc.sync.dma_start(consts[:], consts_d[:])
            ident = consts[:, 0:128]
            onesall = consts[:, 128:256]
            maska = consts[:, 256:512]
            hindt = consts[:, 512:514]

            wq_sb = wp.tile([128, D_MODEL], BF)
            nc.sync.dma_start(wq_sb[:], wq_d[:])
            wk_sb = wp.tile([128, D_MODEL], BF)
            nc.sync.dma_start(wk_sb[:], wk_d[:])
            wvk_sb = wp.tile([128, 2 * D_MODEL], BF)
            nc.sync.dma_start(wvk_sb[:], wvk_d[:])
            wo_sb = wp.tile([128, D_MODEL], BF)
            nc.sync.dma_start(wo_sb[:], wo_d[:])

            xts = {}
            for n0 in range(NT):
                for k in range(KT):
                    xtile = actp.tile([128, 512], BF, tag=f"xt{k}_{n0}", name=f"xt{k}_{n0}")
                    nc.sync.dma_start(
                        xtile[:], xt_d[k * 128 : (k + 1) * 128, n0 * 512 : (n0 + 1) * 512]
                    )
                    xts[(k, n0)] = xtile

            # persistent activations
            qh = [actp.tile([65, N], BF, tag=f"qh{h}", name=f"qh{h}") for h in range(HPC)]
            kh = [actp.tile([65, N], BF, tag=f"kh{h}", name=f"kh{h}") for h in range(HPC)]
            vht = actp.tile([128, N], BF, tag="vht")
            nrmbuf = actp.tile([2, 2 * NT], F32, tag="nrmbuf")
            krows = [actp.tile([128, (N // CH) * RST], BF, tag=f"krows{h}", name=f"krows{h}") for h in range(HPC)]
            vrows = [actp.tile([128, (N // CH) * RST], BF, tag=f"vrows{h}", name=f"vrows{h}") for h in range(HPC)]
            for h in range(HPC):
                nc.sync.dma_start(qh[h][64:65, :], onesrow_d[:])
                nc.sync.dma_start(kh[h][64:65, :], onesrow_d[:])
                nc.gpsimd.memset(krows[h][:], 1.0)
                nc.gpsimd.memset(vrows[h][:], 1.0)

            # ================= projections + norm stats =================
            # qT,kT: transposed layout [65, N]; k,v: token-major rows.
            with (
                tc.tile_pool(name="projps", bufs=2, space="PSUM") as pps,
                tc.tile_pool(name="nrmps", bufs=1, space="PSUM") as nps,
                tc.tile_pool(name="rowpsB", bufs=2, space="PSUM") as rpsB,
                tc.tile_pool(name="sq", bufs=2) as sqp,
            ):
                toggle = 0
                # phase A: q,k projections (unblocks norms asap)
                for n0 in range(NT):
                    cs = slice(n0 * 512, (n0 + 1) * 512)
                    for name, wsb, dsts in (("q", wq_sb, qh), ("k", wk_sb, kh)):
                        p = pps.tile([128, 512], F32, tag=f"p{name}", name=f"p{name}")
                        for k in range(KT):
                            nc.tensor.matmul(
                                p[:],
                                wsb[:, k * 128 : (k + 1) * 128],
                                xts[(k, n0)][:],
                                start=(k == 0),
                                stop=(k == KT - 1),
                            )
                        for h in range(HPC):
                            src = p[h * 64 : (h + 1) * 64, :]
                            if toggle % 2 == 0:
                                nc.vector.tensor_copy(dsts[h][0:64, cs], src)
                            else:
                                nc.scalar.copy(dsts[h][0:64, cs], src)
                            toggle += 1
                        j = 0 if name == "q" else 1
                        sq = sqp.tile([128, 512], BF)
                        nc.scalar.activation(sq[:], p[:], AF.Square)
                        nrm = nps.tile([2, 512], F32)
                        nc.tensor.matmul(nrm[:], hindt, sq[:], start=True, stop=True)
                        nc.vector.tensor_reduce(
                            nrmbuf[:, j * NT + n0 : j * NT + n0 + 1],
                            nrm[:],
                            AX.X,
                            AluOpType.max,
                        )
                # phase B: fused v|k token-major row projections (N=256)
                for sp2 in range(NSPAN):
                    for tok in (2 * sp2, 2 * sp2 + 1):
                        n0, ts = tok // 4, tok % 4
                        rp = rpsB.tile([128, 256], F32, tag="rpvk", name="rpvk")
                        for k in range(KT):
                            nc.tensor.matmul(
                                rp[:],
                                xts[(k, n0)][:, ts * 128 : (ts + 1) * 128],
                                wvk_sb[:, k * 256 : (k + 1) * 256],
                                start=(k == 0),
                                stop=(k == KT - 1),
                            )
                        for hv, rows in ((0, vrows), (1, krows)):
                            for h in range(HPC):
                                dst = rows[h][:, tok * RST : tok * RST + 64]
                                s_ = rp[:, hv * 128 + h * 64 : hv * 128 + (h + 1) * 64]
                                if (tok + hv + h) % 2 == 0:
                                    nc.vector.tensor_copy(dst, s_)
                                else:
                                    nc.scalar.copy(dst, s_)

            # ================= finalize norms -> sqrt(s) per head =================
            with (
                tc.tile_pool(name="nrmfin", bufs=1) as nf,
                tc.tile_pool(name="scps", bufs=2, space="PSUM") as scps,
            ):
                mq = nf.tile([2, 1], F32)
                mk = nf.tile([2, 1], F32)
                nc.vector.tensor_reduce(mq[:], nrmbuf[:, 0:NT], AX.X, AluOpType.max)
                nc.vector.tensor_reduce(mk[:], nrmbuf[:, NT : 2 * NT], AX.X, AluOpType.max)
                prod = nf.tile([2, 1], F32)
                nc.vector.tensor_mul(prod[:], mq[:], mk[:])
                rt = nf.tile([2, 1], F32)
                nc.scalar.activation(rt[:], prod[:], AF.Sqrt)
                rs = nf.tile([2, 1], F32)
                nc.vector.reciprocal(rs[:], rt[:])  # rs = s = 1/(qn*kn)
                rsb = nf.tile([2, 1], BF)
                nc.vector.tensor_copy(rsb[:], rs[:])
                rsh = [nf.tile([1, 1], BF, tag=f"rsh{h}", name=f"rsh{h}") for h in range(HPC)]
                nc.vector.tensor_copy(rsh[0][:], rsb[0:1, :])
                nc.gpsimd.dma_start(rsh[1][:], rsb[1:2, :])
                scv = []  # [64,1] per head (for qh scaling, partition=dim)
                for h in range(HPC):
                    sp64 = scps.tile([64, 1], F32, tag="sp64", name="sp64")
                    nc.tensor.matmul(sp64[:], onesall[0:1, 0:64], rsh[h][:], start=True, stop=True)
                    sv = nf.tile([64, 1], F32, tag=f"scv{h}", name=f"scv{h}")
                    nc.vector.tensor_copy(sv[:], sp64[:])
                    scv.append(sv)
                # fold the whole s into q (k stays unscaled everywhere)
                for h in range(HPC):
                    for n0 in range(NT):
                        cs = slice(n0 * 512, (n0 + 1) * 512)
                        nc.vector.tensor_scalar_mul(qh[h][0:64, cs], qh[h][0:64, cs], scv[h][:])

            # ===== attention (span-major, heads interleaved) + fused outproj =====
            with (
                tc.tile_pool(name="sps", bufs=1, space="PSUM") as sps,
                tc.tile_pool(name="ptps", bufs=1, space="PSUM") as ptps,
                tc.tile_pool(name="ops", bufs=2, space="PSUM") as ops,
                tc.tile_pool(name="vtps", bufs=1, space="PSUM") as vtps,
                tc.tile_pool(name="vtps", bufs=1, space="PSUM") as vtps,
                tc.tile_pool(name="mt", bufs=6) as mtp,
                tc.tile_pool(name="ssb", bufs=3) as ssbp,
                tc.tile_pool(name="recp", bufs=4) as recp,
                tc.tile_pool(name="vhrp", bufs=3) as vhrp,
                tc.tile_pool(name="opps", bufs=2, space="PSUM") as opps,
                tc.tile_pool(name="osb", bufs=4) as osbp,
            ):
                s_chain = {}  # f32 running state per head
                s_snap = {}  # bf16 snapshot consumed by O matmuls
                tgl = 0

                def sweep_step(sp):
                    # span-delta state in a transient bank; chain in SBUF f32
                    for h in range(HPC):
                        ca, cb = 2 * sp, 2 * sp + 1
                        kra = krows[h][:, ca * RST : ca * RST + 65]
                        krb = krows[h][:, cb * RST : cb * RST + 65]
                        vra = vrows[h][:, ca * RST : ca * RST + 65]
                        vrb = vrows[h][:, cb * RST : cb * RST + 65]
                        dl = sps.tile([65, 65], F32, tag="sdelta", name="sdelta", bufs=2)
                        nc.tensor.matmul(dl[:], kra, vra, start=True, stop=False)
                        nc.tensor.matmul(dl[:], krb, vrb, start=False, stop=True)
                        ch = ssbp.tile([65, 65], F32, tag=f"sch{h}_{sp}", name=f"sch{h}_{sp}", bufs=1)
                        if sp == 0:
                            nc.vector.tensor_copy(ch[:], dl[:])
                        else:
                            nc.vector.tensor_add(ch[:], dl[:], s_chain[(h, sp - 1)][:])
                        s_chain[(h, sp)] = ch
                        s_sb = ssbp.tile([65, 65], BF, tag=f"ssb{h}_{sp}", name=f"ssb{h}_{sp}", bufs=1)
                        nc.scalar.copy(s_sb[:], ch[:])
                        s_snap[(h, sp)] = s_sb

                def attention_span(sp):
                    nonlocal tgl
                    qs = slice(sp * SPAN, (sp + 1) * SPAN)
                    cka = slice(sp * SPAN, sp * SPAN + CH)
                    ckb = slice(sp * SPAN + CH, (sp + 1) * SPAN)
                    ca, cb = 2 * sp, 2 * sp + 1
                    vhrs = {ca: vhrp.tile([128, 128], BF, tag="vhra", name="vhra"),
                            cb: vhrp.tile([128, 128], BF, tag="vhrb", name="vhrb")}
                    for h in range(HPC):
                        vra = vrows[h][:, ca * RST : ca * RST + 65]
                        vrb = vrows[h][:, cb * RST : cb * RST + 65]
                        ptj = ptps.tile([128, SPAN + CH], F32, tag="ptj", name="ptj")
                        nc.tensor.matmul(ptj[:, 0:SPAN], kh[h][:, cka], qh[h][:, qs], start=True, stop=True)
                        mta = mtp.tile([128, SPAN], BF, tag="mta", name="mta")
                        nc.vector.tensor_mul(mta[:], ptj[:, 0:SPAN], maska)
                        nc.tensor.matmul(ptj[:, SPAN:], kh[h][:, ckb], qh[h][:, ckb], start=True, stop=True)
                        mtb = mtp.tile([128, CH], BF, tag="mtb", name="mtb")
                        nc.vector.tensor_mul(mtb[:], ptj[:, SPAN:], maska[:, 0:CH])
                        for cidx, ck, mlist in (
                            (ca, cka, [(mta[:, 0:CH], vra)]),
                            (cb, ckb, [(mta[:, CH:SPAN], vra), (mtb[:], vrb)]),
                        ):
                            o = ops.tile([128, 65], F32, tag="ops", name="ops")
                            nmm = len(mlist) + (1 if sp > 0 else 0)
                            for mi, (mm, vv) in enumerate(mlist):
                                nc.tensor.matmul(o[:], mm, vv, start=(mi == 0), stop=(mi == nmm - 1))
                            if sp > 0:
                                nc.tensor.matmul(o[:], qh[h][:, ck], s_snap[(h, sp - 1)][:], start=False, stop=True)
                            rec = recp.tile([128, 1], F32, tag="rec", name="rec")
                            nc.vector.reciprocal(rec[:], o[:, 64:65])
                            rec_b = bass.AP(rec[:].tensor, rec[:].offset, [[1, 128], [0, 64]])
                            nc.vector.tensor_mul(vhrs[cidx][:, h * 64 : (h + 1) * 64], o[:, 0:64], rec_b)
                    for cidx, ck in ((ca, cka), (cb, ckb)):
                        vtp = vtps.tile([128, CH], BF, tag="vtp", name="vtp")
                        nc.tensor.transpose(vtp[:], vhrs[cidx][:], ident)
                        nc.vector.tensor_copy(vht[:, ck], vtp[:])

                def outproj_pair(sp):
                    split = sp == NSPAN - 1  # last pair: spread copies over both engines
                    for r in (2 * sp, 2 * sp + 1):
                        rs_ = slice(r * CH, (r + 1) * CH)
                        ob = osbp.tile([128, D_MODEL], BF, tag="osb", name="osb")
                        for n2 in range(D_MODEL // 512):
                            ns = slice(n2 * 512, (n2 + 1) * 512)
                            op = opps.tile([128, 512], F32, tag="opps", name="opps")
                            nc.tensor.matmul(op[:], vht[:, rs_], wo_sb[:, ns], start=True, stop=True)
                            if split and (r + n2) % 2 == 0:
                                nc.vector.tensor_copy(ob[:, ns], op[:])
                            else:
                                nc.scalar.copy(ob[:, ns], op[:])
                        nc.sync.dma_start(out_d[rs_, :], ob[:])

                # software pipeline: rows(sp) | sweep(sp) | attention(sp-1) | outproj(sp-2)
                for sp in range(NSPAN):
                    if sp < NSPAN - 1:
                        sweep_step(sp)
                    attention_span(sp)
                    if sp >= 1:
                        outproj_pair(sp - 1)
                outproj_pair(NSPAN - 1)

    nc.compile()
    return nc


def _consts():
    import ml_dtypes

    bf = ml_dtypes.bfloat16
    consts = np.zeros((128, 514), dtype=np.float32)
    consts[:, 0:128] = np.eye(128)
    consts[:, 128:256] = 1.0
    j = np.arange(128)[:, None]
    i = np.arange(SPAN)[None, :]
    consts[:, 256:512] = j <= i
    for h in range(HPC):
        consts[h * 64 : (h + 1) * 64, 512 + h] = 1.0
    onesrow = np.ones((1, N), dtype=bf)
    return consts.astype(bf), onesrow


def _in_maps(inputs):
    import ml_dtypes

    bf = ml_dtypes.bfloat16
    X = np.ascontiguousarray(np.asarray(inputs["X"], dtype=np.float32))
    xt = np.ascontiguousarray(X[0].T).astype(bf)  # [D_MODEL, N]
    wqt = np.ascontiguousarray(np.asarray(inputs["Wq"], np.float32).T).astype(bf)
    wkt = np.ascontiguousarray(np.asarray(inputs["Wk"], np.float32).T).astype(bf)
    wvt = np.ascontiguousarray(np.asarray(inputs["Wv"], np.float32).T).astype(bf)
    wot = np.ascontiguousarray(np.asarray(inputs["Wo"], np.float32).T).astype(bf)
    consts, onesrow = _consts()

    def sb_layout(w):  # [1024, 128] -> [128, 8*128] (dm-chunk on partitions)
        return np.ascontiguousarray(
            w.reshape(KT, 128, DPC).transpose(1, 0, 2).reshape(128, KT * DPC)
        )

    def sb_layout2(wv, wk):  # fused [128, 8*256]: per k-chunk [wv128 | wk128]
        a = wv.reshape(KT, 128, DPC).transpose(1, 0, 2)
        b = wk.reshape(KT, 128, DPC).transpose(1, 0, 2)
        return np.ascontiguousarray(
            np.concatenate([a, b], axis=2).reshape(128, KT * 2 * DPC)
        )

    in_maps = []
    for c in range(NCORES):
        cs = slice(c * DPC, (c + 1) * DPC)
        in_maps.append(
            {
                "xt": xt,
                "wq": sb_layout(wqt[:, cs]),
                "wk": sb_layout(wkt[:, cs]),
                "wvk": sb_layout2(wvt[:, cs], wkt[:, cs]),
                "wo": np.ascontiguousarray(wot[cs, :]),
                "consts": consts,
                "onesrow": onesrow,
            }
        )
    return in_maps


def _run(inputs, trace=False):
    from concourse.bass_utils import run_bass_kernel_spmd

    if "nc" not in _CACHE:
        _CACHE["nc"] = _build()
    nc = _CACHE["nc"]
    in_maps = _in_maps(inputs)
    res = run_bass_kernel_spmd(nc, in_maps, core_ids=list(range(NCORES)), trace=trace)
    bo = np.asarray(inputs["bo"], dtype=np.float32)
    acc = np.zeros((N, D_MODEL), dtype=np.float32)
    for c in range(NCORES):
        acc += res.results[c]["out"].astype(np.float32)
    acc += bo[None, :]
    return acc.reshape(B, N, D_MODEL), res.exec_time_ns


def kernel(**inputs) -> np.ndarray:
    out, _ = _run(inputs, trace=False)
    return out

